# revision 5
# baseline (speedup 1.0000x reference)
"""DMoN GCN (3-layer) Trainium2 kernel over 8 NeuronCores — single SPMD launch.

Sharding: core c of 8 handles edges with source in pair-slice s=c//2 (25088
padded nodes, int16-indexable) and target owned by cores of parity g=c%2.
Per layer: PE transform -> pair AllGather of the bf16 node table ->
dma_gather of source rows (4096-idx multi-packet calls rotated over 4 SWDGE
queues so ring drains overlap generation) + PE matmuls against host-shipped
weighted one-hot masks (stored slot-partition-major for contiguous DMA),
accumulating exact segment sums in PSUM -> bf16 partials -> two half-sized
4-way ReduceScatter(add) collectives (block order interleaved (b, rank) so
the first RS overlaps the second half of aggregation) -> fused combine +
selu + next-layer transform in 4-tile batches (softmax at the end).
gcn_norm weights and the self loops are baked into the masks on the host.
A tiny PE "heater" matmul rides every gather so HAM stays warm.
"""

import numpy as np
import ml_dtypes

BF16 = ml_dtypes.bfloat16

N = 100000
IN_DIM = 256
HID = 128
HID2 = 64
K = 16

NCORE = 8
NOWN = 12544            # 98*128 padded nodes per core
NPAIR = 2 * NOWN        # 25088 source rows per pair  (< 32768 -> int16)
NTGT = 4 * NOWN         # 50176 targets per parity group
NBLK = NTGT // 128      # 392 target blocks
NBH = 49                # blocks per rank-quarter half (98 = 2*49)
CHUNK = 8192            # slots per chunk (2 gather calls of 4096)
SUB = CHUNK // 128
GCALL = 4096            # idxs per dma_gather call

SELU_L = 1.0507009873554805
SELU_A = 1.6732632423543772

_CACHE = {}


def _pad_id(r):
    c = r // 12500
    return c * NOWN + (r - c * 12500)


def _build_plan(edge_index, edge_weight):
    row = np.asarray(edge_index[0], dtype=np.int64)
    col = np.asarray(edge_index[1], dtype=np.int64)
    w = np.asarray(edge_weight, dtype=np.float64)

    deg = np.zeros(N, np.float64)
    np.add.at(deg, col, w)
    deg += 1.0                       # self-loop weight
    dinv = np.where(deg > 0, 1.0 / np.sqrt(deg), 0.0)

    # self loops ride the masks as ordinary slots (weight dinv^2)
    loop = np.arange(N, dtype=np.int64)
    row = np.concatenate([row, loop])
    col = np.concatenate([col, loop])
    nw = np.concatenate([dinv[edge_index[0]] * w * dinv[edge_index[1]],
                         dinv * dinv])

    rp = _pad_id(row)
    cp = _pad_id(col)
    src_pair = rp // NPAIR                 # 0..3
    tgt_core = cp // NOWN                  # owner core
    tgt_par = tgt_core % 2                 # parity group
    # local target index within its parity group: rank r = core//2
    tloc = (tgt_core // 2) * NOWN + (cp - tgt_core * NOWN)

    # aggregation processes blocks in (b, rank) interleaved order so the
    # first-half RS (b < 49) can fire at ~50% of the aggregation
    per_core = []
    for c in range(NCORE):
        s, g = c // 2, c % 2
        sel = (src_pair == s) & (tgt_par == g)
        er = (rp[sel] - s * NPAIR).astype(np.int64)
        ec = tloc[sel]
        ew = nw[sel]
        blk = ec // 128
        p = (blk % 98) * 4 + (blk // 98)   # ordered position
        o = np.argsort(p, kind="stable")
        er, ec, ew, p = er[o], ec[o], ew[o], p[o]
        cnt = np.bincount(p, minlength=NBLK)
        pcnt = np.maximum(((cnt + 127) // 128) * 128, 128)
        per_core.append((er, ec, ew, p, cnt, pcnt))

    # shared program structure: per-block sub-chunk counts = max across cores
    nsub_blk = np.stack([pc[5] // 128 for pc in per_core]).max(axis=0)
    tot_sub = int(nsub_blk.sum())
    nchunk = (tot_sub + SUB - 1) // SUB
    pad_sub = nchunk * SUB - tot_sub
    nsub_blk = nsub_blk.copy()
    nsub_blk[-1] += pad_sub
    tot_sub = nchunk * SUB
    sub_off = np.concatenate([[0], np.cumsum(nsub_blk)])[:-1]
    blk_of_sub = np.repeat(np.arange(NBLK), nsub_blk)
    start_of_sub = np.zeros(tot_sub, bool)
    start_of_sub[sub_off] = True

    idx_all, msk_all = [], []
    for c in range(NCORE):
        er, ec, ew, p, cnt, _ = per_core[c]
        nslot = tot_sub * 128
        sidx = np.zeros(nslot, np.int16)
        stgt = np.zeros(nslot, np.int64)
        sw = np.zeros(nslot)
        cnt_off = np.concatenate([[0], np.cumsum(cnt)])[:-1]
        pos = sub_off[p] * 128 + (np.arange(len(ec)) - cnt_off[p])
        sidx[pos] = er.astype(np.int16)
        stgt[pos] = ec % 128
        sw[pos] = ew
        idx_w = np.zeros((nchunk, 128, CHUNK // 16), np.int16)
        ii = np.arange(CHUNK)
        for ch in range(nchunk):
            seg = sidx[ch * CHUNK:(ch + 1) * CHUNK]
            t16 = np.zeros((16, CHUNK // 16), np.int16)
            t16[ii % 16, ii // 16] = seg
            idx_w[ch] = np.tile(t16, (8, 1))
        # slot-partition-major masks: [lane, sub, tgt] for contiguous
        # per-partition chunk reads
        masks = np.zeros((128, tot_sub, 128), np.float32)
        sl = np.arange(nslot)
        masks[sl % 128, sl // 128, stgt] = sw
        idx_all.append(idx_w)
        msk_all.append(masks.astype(BF16))

    return dict(nchunk=nchunk, blk_of_sub=blk_of_sub,
                start_of_sub=start_of_sub, idx=idx_all, msk=msk_all)


def _build_program(nchunk, blk_of_sub, start_of_sub):
    import concourse.bacc as bacc
    import concourse.mybir as mybir
    from concourse import tile

    nc = bacc.Bacc("TRN2", target_bir_lowering=False, debug=False,
                   num_devices=NCORE, num_swdge_queues=4)
    f32, bf16, i16 = mybir.dt.float32, mybir.dt.bfloat16, mybir.dt.int16
    AL = mybir.AluOpType
    TOT_SUB = nchunk * SUB
    FD = [HID, HID2, K]
    NT = NOWN // 128
    NQH = NBH * 128        # 6272 rows per rank-quarter half
    G = 4                  # combine tile-batch

    t_x = nc.dram_tensor("x", [NOWN, IN_DIM], f32, kind="ExternalInput")
    t_idx1 = nc.dram_tensor("idx", [nchunk, 128, CHUNK // 16], i16,
                            kind="ExternalInput")
    t_msk1 = nc.dram_tensor("msk", [128, TOT_SUB, 128], bf16,
                            kind="ExternalInput")
    t_idx = [t_idx1] * 3
    t_msk = [t_msk1] * 3
    t_W0 = nc.dram_tensor("W0p", [2, 128, HID], f32, kind="ExternalInput")
    t_P0 = nc.dram_tensor("P0p", [2, 128, HID], f32, kind="ExternalInput")
    t_W1 = nc.dram_tensor("W1p", [1, 128, HID2], f32, kind="ExternalInput")
    t_P1 = nc.dram_tensor("P1p", [1, 128, HID2], f32, kind="ExternalInput")
    t_W2 = nc.dram_tensor("W2p", [1, 128, K], f32, kind="ExternalInput")
    t_b0 = nc.dram_tensor("b0r", [128, HID], f32, kind="ExternalInput")
    t_b1 = nc.dram_tensor("b1r", [128, HID2], f32, kind="ExternalInput")
    t_b2 = nc.dram_tensor("b2r", [128, K], f32, kind="ExternalInput")
    t_id = nc.dram_tensor("ident", [128, 128], f32, kind="ExternalInput")
    t_out = nc.dram_tensor("out", [NOWN, K], f32, kind="ExternalOutput")

    qctr = [0]
    ectr = [0]

    with tile.TileContext(nc) as tc:
        with (
            tc.tile_pool(name="dram", bufs=1, space="DRAM") as dpool,
            tc.tile_pool(name="wts", bufs=1) as wpool,
            tc.tile_pool(name="work", bufs=4) as pool,
            tc.tile_pool(name="cmb", bufs=3) as cpool,
            tc.tile_pool(name="gbuf", bufs=4) as gpool,
            tc.tile_pool(name="mbuf", bufs=2) as mpool,
            tc.tile_pool(name="ps", bufs=1, space="PSUM") as ppool,
            tc.tile_pool(name="acc", bufs=2, space="PSUM") as apool,
            tc.tile_pool(name="hps", bufs=1, space="PSUM") as hpool,
        ):
            d_tab_own = [dpool.tile([NOWN, 128], bf16, tag=f"tabown{l}", name=f"tabown{l}")
                         for l in range(3)]
            d_tab_pair = [dpool.tile([NPAIR, 128], bf16, tag=f"tabpair{l}", name=f"tabpair{l}")
                          for l in range(3)]
            d_sk = [dpool.tile([NOWN, FD[l]], f32, tag=f"sk{l}", name=f"sk{l}")
                    for l in range(2)]
            d_part_a = [dpool.tile([4 * NQH, FD[l]], bf16, tag=f"parta{l}", name=f"parta{l}")
                        for l in range(3)]
            d_part_b = [dpool.tile([4 * NQH, FD[l]], bf16, tag=f"partb{l}", name=f"partb{l}")
                        for l in range(3)]
            d_rs_a = [dpool.tile([NQH, FD[l]], bf16, tag=f"rsa{l}", name=f"rsa{l}")
                      for l in range(3)]
            d_rs_b = [dpool.tile([NQH, FD[l]], bf16, tag=f"rsb{l}", name=f"rsb{l}")
                      for l in range(3)]

            idt = wpool.tile([128, 128], f32)
            nc.sync.dma_start(idt[:], t_id[:])
            heat_w = wpool.tile([128, 8], bf16)
            nc.vector.tensor_copy(heat_w[:], idt[:, :8])
            def wload(tname, src_t, n, fdim):
                ts = []
                for j in range(n):
                    wt = wpool.tile([128, fdim], f32, name=f"{tname}_{j}")
                    nc.sync.dma_start(wt[:], src_t[j])
                    ts.append(wt)
                return ts
            wW0 = wload("wW0", t_W0, 2, HID)
            wP0 = wload("wP0", t_P0, 2, HID)
            wW1 = wload("wW1", t_W1, 1, HID2)
            wP1 = wload("wP1", t_P1, 1, HID2)
            wW2 = wload("wW2", t_W2, 1, K)
            wb0 = wpool.tile([128, HID], f32)
            nc.sync.dma_start(wb0[:], t_b0[:])
            wb1 = wpool.tile([128, HID2], f32)
            nc.sync.dma_start(wb1[:], t_b1[:])
            wb2 = wpool.tile([128, K], f32)
            nc.sync.dma_start(wb2[:], t_b2[:])
            # bias for the batched final-layer combine: [128, G, K]
            wb2g = wpool.tile([128, G, K], f32)
            for j in range(G):
                nc.vector.tensor_copy(wb2g[:, j, :], wb2[:, :K])

            def transform0():
                fin, fout = IN_DIM, HID
                ncin = fin // 128
                for t in range(NT):
                    xin = pool.tile([128, fin], f32, tag="xin")
                    nc.sync.dma_start(xin[:],
                                      t_x[t * 128:(t + 1) * 128, :])
                    xT = pool.tile([128, fin], f32, tag="xT")
                    for j in range(ncin):
                        pt = ppool.tile([128, 128], f32, tag="ptr")
                        nc.tensor.transpose(
                            pt[:], xin[:, j * 128:(j + 1) * 128], idt[:])
                        nc.vector.tensor_copy(
                            xT[:, j * 128:(j + 1) * 128], pt[:])
                    pm = ppool.tile([128, fout], f32, tag="pmm")
                    for j in range(ncin):
                        nc.tensor.matmul(pm[:], xT[:, j * 128:(j + 1) * 128],
                                         wW0[j][:], start=(j == 0),
                                         stop=(j == ncin - 1))
                    tb = pool.tile([128, 128], bf16, tag="tabtile")
                    nc.vector.tensor_copy(tb[:], pm[:])
                    nc.sync.dma_start(
                        d_tab_own[0][t * 128:(t + 1) * 128, :], tb[:])
                    ps = ppool.tile([128, fout], f32, tag="psk")
                    for j in range(ncin):
                        nc.tensor.matmul(ps[:],
                                         xT[:, j * 128:(j + 1) * 128],
                                         wP0[j][:], start=(j == 0),
                                         stop=(j == ncin - 1))
                    sk = pool.tile([128, fout], f32, tag="sktile")
                    nc.vector.tensor_tensor(sk[:], ps[:], wb0[:], AL.add)
                    nc.sync.dma_start(
                        d_sk[0][t * 128:(t + 1) * 128, :], sk[:])

            def aggregate(l, fout):
                acc = None
                sub = 0
                half_a_done = False
                for ch in range(nchunk):
                    it = pool.tile([128, CHUNK // 16], i16, tag="idxt")
                    nc.sync.dma_start(it[:], t_idx[l][ch])
                    g = gpool.tile([128, SUB, 128], bf16, tag="gath")
                    NS = CHUNK // GCALL          # 2 calls of 4096
                    SL = GCALL
                    for k in range(NS):
                        nc.gpsimd.dma_gather(
                            g[:, k * (SUB // NS):(k + 1) * (SUB // NS), :],
                            d_tab_pair[l][:],
                            it[:, k * (SL // 16):(k + 1) * (SL // 16)],
                            SL, SL, 128,
                            single_packet=False,
                            queue_num=qctr[0] % 4)
                        qctr[0] += 1
                        # HAM heater: tiny MM dependent on this gather's data
                        hp = hpool.tile([8, 8], f32, tag="heat")
                        nc.tensor.matmul(hp[:], heat_w[:, :8],
                                         g[:, k * (SUB // NS), :8],
                                         start=True, stop=True)
                    mk = mpool.tile([128, SUB, 128], bf16, tag="maskt")
                    # scalar-engine HWDGE ring: keeps the mask stream off the
                    # sync ring; contiguous 16KB per partition per chunk
                    nc.scalar.dma_start(
                        mk[:], t_msk[l][:, ch * SUB:(ch + 1) * SUB, :])
                    for j in range(SUB):
                        st = bool(start_of_sub[sub])
                        if st:
                            acc = apool.tile([128, fout], f32, tag="accps",
                                             name=f"acc{l}_{sub}")
                        last = (sub == TOT_SUB - 1) or bool(
                            start_of_sub[sub + 1])
                        nc.tensor.matmul(acc[:], mk[:, j, :],
                                         g[:, j, :fout], start=st, stop=last)
                        if last:
                            p = int(blk_of_sub[sub])
                            r, b = p % 4, p // 4
                            ev = pool.tile([128, fout], bf16, tag="ev")
                            nc.vector.tensor_copy(ev[:], acc[:])
                            if b < NBH:
                                dst = d_part_a[l]
                                row = r * NQH + b * 128
                            else:
                                dst = d_part_b[l]
                                row = r * NQH + (b - NBH) * 128
                            # alternate HWDGE rings for the partial writes
                            eng = nc.sync if ectr[0] % 2 == 0 else nc.scalar
                            ectr[0] += 1
                            eng.dma_start(dst[row:row + 128, :], ev[:])
                            if b == NBH - 1 and r == 3 and not half_a_done:
                                half_a_done = True
                                nc.gpsimd.collective_compute(
                                    "ReduceScatter", mybir.AluOpType.add,
                                    replica_groups=[[0, 2, 4, 6],
                                                    [1, 3, 5, 7]],
                                    ins=[d_part_a[l][:].opt()],
                                    outs=[d_rs_a[l][:].opt()])
                        sub += 1
                nc.gpsimd.collective_compute(
                    "ReduceScatter", mybir.AluOpType.add,
                    replica_groups=[[0, 2, 4, 6], [1, 3, 5, 7]],
                    ins=[d_part_b[l][:].opt()], outs=[d_rs_b[l][:].opt()])

            def load_rs_group(l, t0, gg, fout):
                """Load gg tiles t0..t0+gg-1 of the RS result (bf16)."""
                rsb = cpool.tile([128, G, fout], bf16, tag="rsld")
                j = 0
                while j < gg:
                    t = t0 + j
                    if t < NBH:
                        run = min(gg - j, NBH - t)
                        src = d_rs_a[l][t * 128:(t + run) * 128, :]
                    else:
                        run = gg - j
                        src = d_rs_b[l][(t - NBH) * 128:
                                        (t - NBH + run) * 128, :]
                    nc.sync.dma_start(
                        rsb[:, j:j + run, :],
                        src.rearrange("(a p) f -> p a f", p=128))
                    j += run
                return rsb

            def combine(l, fout):
                """rs + skip + selu; for l<2 also the next-layer transform
                (tab/skip tiles for layer l+1) fused per 4-tile group."""
                fo2 = FD[l + 1] if l < 2 else 0
                for t0 in range(0, NT, G):
                    gg = min(G, NT - t0)
                    rsb = load_rs_group(l, t0, gg, fout)
                    zz = cpool.tile([128, G, fout], f32, tag="z")
                    nc.vector.tensor_copy(zz[:, :gg, :], rsb[:, :gg, :])
                    if l < 2:
                        sk = cpool.tile([128, G, fout], f32, tag="skld")
                        nc.sync.dma_start(
                            sk[:, :gg, :],
                            d_sk[l][t0 * 128:(t0 + gg) * 128, :]
                            .rearrange("(a p) f -> p a f", p=128))
                        nc.vector.tensor_tensor(zz[:, :gg, :], zz[:, :gg, :],
                                                sk[:, :gg, :], AL.add)
                    else:
                        nc.vector.tensor_tensor(zz[:, :gg, :], zz[:, :gg, :],
                                                wb2g[:, :gg, :], AL.add)
                    zf = zz[:, :gg, :].rearrange("p a f -> p (a f)")
                    mn = cpool.tile([128, G * fout], f32, tag="smn")
                    nc.vector.tensor_scalar_min(mn[:, :gg * fout], zf, 0.0)
                    ex = cpool.tile([128, G * fout], f32, tag="sex")
                    nc.scalar.activation(ex[:, :gg * fout], mn[:, :gg * fout],
                                         mybir.ActivationFunctionType.Exp)
                    nc.vector.tensor_scalar(ex[:, :gg * fout],
                                            ex[:, :gg * fout],
                                            SELU_L * SELU_A,
                                            -SELU_L * SELU_A, AL.mult,
                                            AL.add)
                    nc.vector.tensor_scalar_max(zf, zf, 0.0)
                    nc.vector.tensor_scalar(zf, zf, SELU_L, None, AL.mult)
                    nc.vector.tensor_tensor(zf, zf, ex[:, :gg * fout],
                                            AL.add)
                    if l < 2:
                        # fused transform for layer l+1 (fin = fout here)
                        Wn = wW1 if l == 0 else wW2
                        Pn = wP1 if l == 0 else None
                        bn = wb1 if l == 0 else None
                        for j in range(gg):
                            t = t0 + j
                            pt = ppool.tile([128, 128], f32, tag="ptr")
                            nc.tensor.transpose(pt[:fout, :], zz[:, j, :],
                                                idt[:])
                            xT = pool.tile([128, 128], f32, tag="xT")
                            nc.vector.tensor_copy(xT[:fout, :], pt[:fout, :])
                            pm = ppool.tile([128, fo2], f32, tag="pmm")
                            nc.tensor.matmul(pm[:], xT[:fout, :], Wn[0][:fout, :],
                                             start=True, stop=True)
                            tb = pool.tile([128, 128], bf16, tag="tabtile")
                            if fo2 < 128:
                                nc.gpsimd.memset(tb[:], 0.0)
                            nc.vector.tensor_copy(tb[:, :fo2], pm[:])
                            nc.sync.dma_start(
                                d_tab_own[l + 1][t * 128:(t + 1) * 128, :],
                                tb[:])
                            if Pn is not None:
                                ps = ppool.tile([128, fo2], f32, tag="psk")
                                nc.tensor.matmul(ps[:], xT[:fout, :],
                                                 Pn[0][:fout, :],
                                                 start=True, stop=True)
                                sk2 = pool.tile([128, fo2], f32,
                                                tag="sktile")
                                nc.vector.tensor_tensor(sk2[:], ps[:],
                                                        bn[:, :fo2], AL.add)
                                nc.sync.dma_start(
                                    d_sk[l + 1][t * 128:(t + 1) * 128, :],
                                    sk2[:])
                    else:
                        # softmax over fout=16 (no max-shift; values bounded)
                        eo = cpool.tile([128, G, fout], f32, tag="soft")
                        nc.scalar.activation(
                            eo[:, :gg, :].rearrange("p a f -> p (a f)"), zf,
                            mybir.ActivationFunctionType.Exp)
                        sm = cpool.tile([128, G], f32, tag="ssm")
                        nc.vector.tensor_reduce(sm[:, :gg], eo[:, :gg, :],
                                                mybir.AxisListType.X, AL.add)
                        rc = cpool.tile([128, G], f32, tag="src")
                        nc.vector.reciprocal(rc[:, :gg], sm[:, :gg])
                        for j in range(gg):
                            t = t0 + j
                            nc.vector.tensor_scalar(eo[:, j, :], eo[:, j, :],
                                                    rc[:, j:j + 1], None,
                                                    AL.mult)
                            nc.sync.dma_start(
                                t_out[t * 128:(t + 1) * 128, :],
                                eo[:, j, :])

            def allgather(l):
                nc.gpsimd.collective_compute(
                    "AllGather", mybir.AluOpType.bypass,
                    replica_groups=[[0, 1], [2, 3], [4, 5], [6, 7]],
                    ins=[d_tab_own[l][:].opt()],
                    outs=[d_tab_pair[l][:].opt()])

            transform0()
            allgather(0)
            for l in range(3):
                aggregate(l, FD[l])
                combine(l, FD[l])
                if l < 2:
                    allgather(l + 1)
    nc.compile()
    return nc


def _get_compiled(inputs):
    k = "prog"
    if k not in _CACHE:
        plan = _build_plan(inputs["edge_index"], inputs["edge_weight"])
        nc = _build_program(plan["nchunk"], plan["blk_of_sub"],
                            plan["start_of_sub"])
        _CACHE[k] = (plan, nc)
    return _CACHE[k]


def kernel(_trace=False, **inputs):
    from concourse.bass_utils import run_bass_kernel_spmd

    plan, nc = _get_compiled(inputs)

    x = np.asarray(inputs["x"], np.float32)
    xpad = np.zeros((NCORE, NOWN, IN_DIM), np.float32)
    for c in range(NCORE):
        xpad[c, :12500] = x[c * 12500:(c + 1) * 12500]

    def wchunks(W, n):
        out = np.zeros((n, 128, W.shape[1]), np.float32)
        for j in range(n):
            out[j, :min(128, W.shape[0] - j * 128)] = \
                W[j * 128:(j + 1) * 128]
        return out

    W0 = np.asarray(inputs["W0"], np.float32)
    P0 = np.asarray(inputs["P0w"], np.float32)
    W1p = wchunks(np.asarray(inputs["W1"], np.float32), 1)
    P1p = wchunks(np.asarray(inputs["P1w"], np.float32), 1)
    W2p = wchunks(np.asarray(inputs["W2"], np.float32), 1)
    b0r = np.tile((np.asarray(inputs["b0"]) + np.asarray(inputs["P0b"]))
                  .astype(np.float32), (128, 1))
    b1r = np.tile((np.asarray(inputs["b1"]) + np.asarray(inputs["P1b"]))
                  .astype(np.float32), (128, 1))
    b2r = np.tile(np.asarray(inputs["b2"]).astype(np.float32), (128, 1))

    in_maps = []
    for c in range(NCORE):
        in_maps.append({
            "x": xpad[c],
            "idx": plan["idx"][c], "msk": plan["msk"][c],
            "W0p": wchunks(W0, 2), "P0p": wchunks(P0, 2),
            "W1p": W1p, "P1p": P1p, "W2p": W2p,
            "b0r": b0r, "b1r": b1r, "b2r": b2r,
            "ident": np.eye(128, dtype=np.float32),
        })
    res = run_bass_kernel_spmd(nc, in_maps, core_ids=list(range(NCORE)),
                               trace=_trace)
    if _trace:
        kernel.last_exec_ns = res.exec_time_ns
    out = np.zeros((N, K), np.float32)
    for c in range(NCORE):
        out[c * 12500:(c + 1) * 12500] = res.results[c]["out"][:12500]
    return out


# revision 6
# speedup vs baseline: 1.0421x; 1.0421x over previous
"""DMoN GCN (3-layer) Trainium2 kernel over 8 NeuronCores — single SPMD launch.

Sharding: core c of 8 handles edges with source in pair-slice s=c//2 (25088
padded nodes, int16-indexable) and target owned by cores of parity g=c%2.
Per layer: PE transform -> pair AllGather of the bf16 node table ->
dma_gather of source rows (4096-idx multi-packet calls rotated over 4 SWDGE
queues so ring drains overlap generation) + PE matmuls against host-shipped
weighted one-hot masks (stored slot-partition-major for contiguous DMA),
accumulating exact segment sums in PSUM -> bf16 partials -> two half-sized
4-way ReduceScatter(add) collectives (block order interleaved (b, rank) so
the first RS overlaps the second half of aggregation) -> fused combine +
selu + next-layer transform in 4-tile batches (softmax at the end).
gcn_norm weights and the self loops are baked into the masks on the host.
A tiny PE "heater" matmul rides every gather so HAM stays warm.
"""

import numpy as np
import ml_dtypes

BF16 = ml_dtypes.bfloat16

N = 100000
IN_DIM = 256
HID = 128
HID2 = 64
K = 16

NCORE = 8
NOWN = 12544            # 98*128 padded nodes per core
NPAIR = 2 * NOWN        # 25088 source rows per pair  (< 32768 -> int16)
NTGT = 4 * NOWN         # 50176 targets per parity group
NBLK = NTGT // 128      # 392 target blocks
NBH = 49                # blocks per rank-quarter half (98 = 2*49)
CHUNK = 8192            # slots per chunk (2 gather calls of 4096)
SUB = CHUNK // 128
GCALL = 4096            # idxs per dma_gather call

SELU_L = 1.0507009873554805
SELU_A = 1.6732632423543772

_CACHE = {}


def _pad_id(r):
    c = r // 12500
    return c * NOWN + (r - c * 12500)


def _build_plan(edge_index, edge_weight):
    row = np.asarray(edge_index[0], dtype=np.int64)
    col = np.asarray(edge_index[1], dtype=np.int64)
    w = np.asarray(edge_weight, dtype=np.float64)

    deg = np.zeros(N, np.float64)
    np.add.at(deg, col, w)
    deg += 1.0                       # self-loop weight
    dinv = np.where(deg > 0, 1.0 / np.sqrt(deg), 0.0)

    # self loops ride the masks as ordinary slots (weight dinv^2)
    loop = np.arange(N, dtype=np.int64)
    row = np.concatenate([row, loop])
    col = np.concatenate([col, loop])
    nw = np.concatenate([dinv[edge_index[0]] * w * dinv[edge_index[1]],
                         dinv * dinv])

    rp = _pad_id(row)
    cp = _pad_id(col)
    src_pair = rp // NPAIR                 # 0..3
    tgt_core = cp // NOWN                  # owner core
    tgt_par = tgt_core % 2                 # parity group
    # local target index within its parity group: rank r = core//2
    tloc = (tgt_core // 2) * NOWN + (cp - tgt_core * NOWN)

    # aggregation processes blocks in (b, rank) interleaved order so the
    # first-half RS (b < 49) can fire at ~50% of the aggregation
    per_core = []
    for c in range(NCORE):
        s, g = c // 2, c % 2
        sel = (src_pair == s) & (tgt_par == g)
        er = (rp[sel] - s * NPAIR).astype(np.int64)
        ec = tloc[sel]
        ew = nw[sel]
        blk = ec // 128
        p = (blk % 98) * 4 + (blk // 98)   # ordered position
        o = np.argsort(p, kind="stable")
        er, ec, ew, p = er[o], ec[o], ew[o], p[o]
        cnt = np.bincount(p, minlength=NBLK)
        pcnt = np.maximum(((cnt + 127) // 128) * 128, 128)
        per_core.append((er, ec, ew, p, cnt, pcnt))

    # shared program structure: per-block sub-chunk counts = max across cores
    nsub_blk = np.stack([pc[5] // 128 for pc in per_core]).max(axis=0)
    tot_sub = int(nsub_blk.sum())
    nchunk = (tot_sub + SUB - 1) // SUB
    pad_sub = nchunk * SUB - tot_sub
    nsub_blk = nsub_blk.copy()
    nsub_blk[-1] += pad_sub
    tot_sub = nchunk * SUB
    sub_off = np.concatenate([[0], np.cumsum(nsub_blk)])[:-1]
    blk_of_sub = np.repeat(np.arange(NBLK), nsub_blk)
    start_of_sub = np.zeros(tot_sub, bool)
    start_of_sub[sub_off] = True

    idx_all, msk_all = [], []
    for c in range(NCORE):
        er, ec, ew, p, cnt, _ = per_core[c]
        nslot = tot_sub * 128
        sidx = np.zeros(nslot, np.int16)
        stgt = np.zeros(nslot, np.int64)
        sw = np.zeros(nslot)
        cnt_off = np.concatenate([[0], np.cumsum(cnt)])[:-1]
        pos = sub_off[p] * 128 + (np.arange(len(ec)) - cnt_off[p])
        sidx[pos] = er.astype(np.int16)
        stgt[pos] = ec % 128
        sw[pos] = ew
        idx_w = np.zeros((nchunk, 128, CHUNK // 16), np.int16)
        ii = np.arange(CHUNK)
        for ch in range(nchunk):
            seg = sidx[ch * CHUNK:(ch + 1) * CHUNK]
            t16 = np.zeros((16, CHUNK // 16), np.int16)
            t16[ii % 16, ii // 16] = seg
            idx_w[ch] = np.tile(t16, (8, 1))
        # slot-partition-major masks: [lane, sub, tgt] for contiguous
        # per-partition chunk reads
        masks = np.zeros((128, tot_sub, 128), np.float32)
        sl = np.arange(nslot)
        masks[sl % 128, sl // 128, stgt] = sw
        idx_all.append(idx_w)
        msk_all.append(masks.astype(BF16))

    return dict(nchunk=nchunk, blk_of_sub=blk_of_sub,
                start_of_sub=start_of_sub, idx=idx_all, msk=msk_all)


def _build_program(nchunk, blk_of_sub, start_of_sub):
    import concourse.bacc as bacc
    import concourse.mybir as mybir
    from concourse import tile

    nc = bacc.Bacc("TRN2", target_bir_lowering=False, debug=False,
                   num_devices=NCORE, num_swdge_queues=4)
    f32, bf16, i16 = mybir.dt.float32, mybir.dt.bfloat16, mybir.dt.int16
    AL = mybir.AluOpType
    TOT_SUB = nchunk * SUB
    FD = [HID, HID2, K]
    NT = NOWN // 128
    NQH = NBH * 128        # 6272 rows per rank-quarter half
    G = 4                  # combine tile-batch

    t_x = nc.dram_tensor("x", [NOWN, IN_DIM], f32, kind="ExternalInput")
    t_idx1 = nc.dram_tensor("idx", [nchunk, 128, CHUNK // 16], i16,
                            kind="ExternalInput")
    t_msk1 = nc.dram_tensor("msk", [128, TOT_SUB, 128], bf16,
                            kind="ExternalInput")
    t_idx = [t_idx1] * 3
    t_msk = [t_msk1] * 3
    t_W0 = nc.dram_tensor("W0p", [2, 128, HID], f32, kind="ExternalInput")
    t_P0 = nc.dram_tensor("P0p", [2, 128, HID], f32, kind="ExternalInput")
    t_W1 = nc.dram_tensor("W1p", [1, 128, HID2], f32, kind="ExternalInput")
    t_P1 = nc.dram_tensor("P1p", [1, 128, HID2], f32, kind="ExternalInput")
    t_W2 = nc.dram_tensor("W2p", [1, 128, K], f32, kind="ExternalInput")
    t_b0 = nc.dram_tensor("b0r", [128, HID], f32, kind="ExternalInput")
    t_b1 = nc.dram_tensor("b1r", [128, HID2], f32, kind="ExternalInput")
    t_b2 = nc.dram_tensor("b2r", [128, K], f32, kind="ExternalInput")
    t_id = nc.dram_tensor("ident", [128, 128], f32, kind="ExternalInput")
    t_out = nc.dram_tensor("out", [NOWN, K], f32, kind="ExternalOutput")

    qctr = [0]
    ectr = [0]

    with tile.TileContext(nc) as tc:
        with (
            tc.tile_pool(name="dram", bufs=1, space="DRAM") as dpool,
            tc.tile_pool(name="wts", bufs=1) as wpool,
            tc.tile_pool(name="work", bufs=4) as pool,
            tc.tile_pool(name="idxp", bufs=8) as ipool,
            tc.tile_pool(name="cmb", bufs=2) as cpool,
            tc.tile_pool(name="gbuf", bufs=5) as gpool,
            tc.tile_pool(name="mbuf", bufs=3) as mpool,
            tc.tile_pool(name="ps", bufs=2, space="PSUM") as ppool,
            tc.tile_pool(name="acc", bufs=2, space="PSUM") as apool,
        ):
            d_tab_own = [dpool.tile([NOWN, 128], bf16, tag=f"tabown{l}", name=f"tabown{l}")
                         for l in range(3)]
            d_tab_pair = [dpool.tile([NPAIR, 128], bf16, tag=f"tabpair{l}", name=f"tabpair{l}")
                          for l in range(3)]
            d_sk = [dpool.tile([NOWN, FD[l]], f32, tag=f"sk{l}", name=f"sk{l}")
                    for l in range(2)]
            d_part_a = [dpool.tile([4 * NQH, FD[l]], bf16, tag=f"parta{l}", name=f"parta{l}")
                        for l in range(3)]
            d_part_b = [dpool.tile([4 * NQH, FD[l]], bf16, tag=f"partb{l}", name=f"partb{l}")
                        for l in range(3)]
            d_rs_a = [dpool.tile([NQH, FD[l]], bf16, tag=f"rsa{l}", name=f"rsa{l}")
                      for l in range(3)]
            d_rs_b = [dpool.tile([NQH, FD[l]], bf16, tag=f"rsb{l}", name=f"rsb{l}")
                      for l in range(3)]

            idt = wpool.tile([128, 128], f32)
            nc.sync.dma_start(idt[:], t_id[:])
            def wload(tname, src_t, n, fdim):
                ts = []
                for j in range(n):
                    wt = wpool.tile([128, fdim], f32, name=f"{tname}_{j}")
                    nc.sync.dma_start(wt[:], src_t[j])
                    ts.append(wt)
                return ts
            wW0 = wload("wW0", t_W0, 2, HID)
            wP0 = wload("wP0", t_P0, 2, HID)
            wW1 = wload("wW1", t_W1, 1, HID2)
            wP1 = wload("wP1", t_P1, 1, HID2)
            wW2 = wload("wW2", t_W2, 1, K)
            wb0 = wpool.tile([128, HID], f32)
            nc.sync.dma_start(wb0[:], t_b0[:])
            wb1 = wpool.tile([128, HID2], f32)
            nc.sync.dma_start(wb1[:], t_b1[:])
            wb2 = wpool.tile([128, K], f32)
            nc.sync.dma_start(wb2[:], t_b2[:])
            # bias for the batched final-layer combine: [128, G, K]
            wb2g = wpool.tile([128, G, K], f32)
            for j in range(G):
                nc.vector.tensor_copy(wb2g[:, j, :], wb2[:, :K])

            def transform0():
                fin, fout = IN_DIM, HID
                ncin = fin // 128
                for t in range(NT):
                    xin = pool.tile([128, fin], f32, tag="xin")
                    nc.sync.dma_start(xin[:],
                                      t_x[t * 128:(t + 1) * 128, :])
                    xT = pool.tile([128, fin], f32, tag="xT")
                    for j in range(ncin):
                        pt = ppool.tile([128, 128], f32, tag="ptr")
                        nc.tensor.transpose(
                            pt[:], xin[:, j * 128:(j + 1) * 128], idt[:])
                        nc.vector.tensor_copy(
                            xT[:, j * 128:(j + 1) * 128], pt[:])
                    pm = ppool.tile([128, fout], f32, tag="pmm")
                    for j in range(ncin):
                        nc.tensor.matmul(pm[:], xT[:, j * 128:(j + 1) * 128],
                                         wW0[j][:], start=(j == 0),
                                         stop=(j == ncin - 1))
                    tb = pool.tile([128, 128], bf16, tag="tabtile")
                    nc.vector.tensor_copy(tb[:], pm[:])
                    nc.sync.dma_start(
                        d_tab_own[0][t * 128:(t + 1) * 128, :], tb[:])
                    ps = ppool.tile([128, fout], f32, tag="psk")
                    for j in range(ncin):
                        nc.tensor.matmul(ps[:],
                                         xT[:, j * 128:(j + 1) * 128],
                                         wP0[j][:], start=(j == 0),
                                         stop=(j == ncin - 1))
                    sk = pool.tile([128, fout], f32, tag="sktile")
                    nc.vector.tensor_tensor(sk[:], ps[:], wb0[:], AL.add)
                    nc.sync.dma_start(
                        d_sk[0][t * 128:(t + 1) * 128, :], sk[:])

            def aggregate(l, fout):
                acc = None
                sub = 0
                half_a_done = False
                for ch in range(nchunk):
                    it = ipool.tile([128, CHUNK // 16], i16, tag="idxt")
                    nc.sync.dma_start(it[:], t_idx[l][ch])
                    g = gpool.tile([128, SUB, 128], bf16, tag="gath")
                    NS = CHUNK // GCALL          # 2 calls of 4096
                    SL = GCALL
                    for k in range(NS):
                        nc.gpsimd.dma_gather(
                            g[:, k * (SUB // NS):(k + 1) * (SUB // NS), :],
                            d_tab_pair[l][:],
                            it[:, k * (SL // 16):(k + 1) * (SL // 16)],
                            SL, SL, 128,
                            single_packet=False,
                            queue_num=qctr[0] % 4)
                        qctr[0] += 1
                    mk = mpool.tile([128, SUB, 128], bf16, tag="maskt")
                    # scalar-engine HWDGE ring: keeps the mask stream off the
                    # sync ring; contiguous 16KB per partition per chunk
                    nc.scalar.dma_start(
                        mk[:], t_msk[l][:, ch * SUB:(ch + 1) * SUB, :])
                    for j in range(SUB):
                        st = bool(start_of_sub[sub])
                        if st:
                            acc = apool.tile([128, fout], f32, tag="accps",
                                             name=f"acc{l}_{sub}")
                        last = (sub == TOT_SUB - 1) or bool(
                            start_of_sub[sub + 1])
                        nc.tensor.matmul(acc[:], mk[:, j, :],
                                         g[:, j, :fout], start=st, stop=last)
                        if last:
                            p = int(blk_of_sub[sub])
                            r, b = p % 4, p // 4
                            ev = pool.tile([128, fout], bf16, tag="ev")
                            nc.vector.tensor_copy(ev[:], acc[:])
                            if b < NBH:
                                dst = d_part_a[l]
                                row = r * NQH + b * 128
                            else:
                                dst = d_part_b[l]
                                row = r * NQH + (b - NBH) * 128
                            # alternate HWDGE rings for the partial writes
                            eng = nc.sync if ectr[0] % 2 == 0 else nc.scalar
                            ectr[0] += 1
                            eng.dma_start(dst[row:row + 128, :], ev[:])
                            if b == NBH - 1 and r == 3 and not half_a_done:
                                half_a_done = True
                                nc.gpsimd.collective_compute(
                                    "ReduceScatter", mybir.AluOpType.add,
                                    replica_groups=[[0, 2, 4, 6],
                                                    [1, 3, 5, 7]],
                                    ins=[d_part_a[l][:].opt()],
                                    outs=[d_rs_a[l][:].opt()])
                        sub += 1
                nc.gpsimd.collective_compute(
                    "ReduceScatter", mybir.AluOpType.add,
                    replica_groups=[[0, 2, 4, 6], [1, 3, 5, 7]],
                    ins=[d_part_b[l][:].opt()], outs=[d_rs_b[l][:].opt()])

            def load_rs_group(l, t0, gg, fout):
                """Load gg tiles t0..t0+gg-1 of the RS result (bf16)."""
                rsb = cpool.tile([128, G, fout], bf16, tag="rsld")
                j = 0
                while j < gg:
                    t = t0 + j
                    if t < NBH:
                        run = min(gg - j, NBH - t)
                        src = d_rs_a[l][t * 128:(t + run) * 128, :]
                    else:
                        run = gg - j
                        src = d_rs_b[l][(t - NBH) * 128:
                                        (t - NBH + run) * 128, :]
                    nc.sync.dma_start(
                        rsb[:, j:j + run, :],
                        src.rearrange("(a p) f -> p a f", p=128))
                    j += run
                return rsb

            def combine(l, fout):
                """rs + skip + selu; for l<2 also the next-layer transform
                (tab/skip tiles for layer l+1) fused per 4-tile group."""
                fo2 = FD[l + 1] if l < 2 else 0
                for t0 in range(0, NT, G):
                    gg = min(G, NT - t0)
                    rsb = load_rs_group(l, t0, gg, fout)
                    zz = cpool.tile([128, G, fout], f32, tag="z")
                    nc.vector.tensor_copy(zz[:, :gg, :], rsb[:, :gg, :])
                    if l < 2:
                        sk = cpool.tile([128, G, fout], f32, tag="skld")
                        nc.sync.dma_start(
                            sk[:, :gg, :],
                            d_sk[l][t0 * 128:(t0 + gg) * 128, :]
                            .rearrange("(a p) f -> p a f", p=128))
                        nc.vector.tensor_tensor(zz[:, :gg, :], zz[:, :gg, :],
                                                sk[:, :gg, :], AL.add)
                    else:
                        nc.vector.tensor_tensor(zz[:, :gg, :], zz[:, :gg, :],
                                                wb2g[:, :gg, :], AL.add)
                    zf = zz[:, :gg, :].rearrange("p a f -> p (a f)")
                    mn = cpool.tile([128, G * fout], f32, tag="smn")
                    nc.vector.tensor_scalar_min(mn[:, :gg * fout], zf, 0.0)
                    ex = cpool.tile([128, G * fout], f32, tag="sex")
                    nc.scalar.activation(ex[:, :gg * fout], mn[:, :gg * fout],
                                         mybir.ActivationFunctionType.Exp)
                    nc.vector.tensor_scalar(ex[:, :gg * fout],
                                            ex[:, :gg * fout],
                                            SELU_L * SELU_A,
                                            -SELU_L * SELU_A, AL.mult,
                                            AL.add)
                    nc.vector.tensor_scalar_max(zf, zf, 0.0)
                    nc.vector.tensor_scalar(zf, zf, SELU_L, None, AL.mult)
                    nc.vector.tensor_tensor(zf, zf, ex[:, :gg * fout],
                                            AL.add)
                    if l < 2:
                        # fused transform for layer l+1 (fin = fout here)
                        Wn = wW1 if l == 0 else wW2
                        Pn = wP1 if l == 0 else None
                        bn = wb1 if l == 0 else None
                        for j in range(gg):
                            t = t0 + j
                            pt = ppool.tile([128, 128], f32, tag="ptr")
                            nc.tensor.transpose(pt[:fout, :], zz[:, j, :],
                                                idt[:])
                            xT = pool.tile([128, 128], f32, tag="xT")
                            nc.vector.tensor_copy(xT[:fout, :], pt[:fout, :])
                            pm = ppool.tile([128, fo2], f32, tag="pmm")
                            nc.tensor.matmul(pm[:], xT[:fout, :], Wn[0][:fout, :],
                                             start=True, stop=True)
                            tb = pool.tile([128, 128], bf16, tag="tabtile")
                            if fo2 < 128:
                                nc.gpsimd.memset(tb[:], 0.0)
                            nc.vector.tensor_copy(tb[:, :fo2], pm[:])
                            nc.sync.dma_start(
                                d_tab_own[l + 1][t * 128:(t + 1) * 128, :],
                                tb[:])
                            if Pn is not None:
                                ps = ppool.tile([128, fo2], f32, tag="psk")
                                nc.tensor.matmul(ps[:], xT[:fout, :],
                                                 Pn[0][:fout, :],
                                                 start=True, stop=True)
                                sk2 = pool.tile([128, fo2], f32,
                                                tag="sktile")
                                nc.vector.tensor_tensor(sk2[:], ps[:],
                                                        bn[:, :fo2], AL.add)
                                nc.sync.dma_start(
                                    d_sk[l + 1][t * 128:(t + 1) * 128, :],
                                    sk2[:])
                    else:
                        # softmax over fout=16 (no max-shift; values bounded)
                        eo = cpool.tile([128, G, fout], f32, tag="soft")
                        nc.scalar.activation(
                            eo[:, :gg, :].rearrange("p a f -> p (a f)"), zf,
                            mybir.ActivationFunctionType.Exp)
                        sm = cpool.tile([128, G], f32, tag="ssm")
                        nc.vector.tensor_reduce(sm[:, :gg], eo[:, :gg, :],
                                                mybir.AxisListType.X, AL.add)
                        rc = cpool.tile([128, G], f32, tag="src")
                        nc.vector.reciprocal(rc[:, :gg], sm[:, :gg])
                        for j in range(gg):
                            t = t0 + j
                            nc.vector.tensor_scalar(eo[:, j, :], eo[:, j, :],
                                                    rc[:, j:j + 1], None,
                                                    AL.mult)
                            nc.sync.dma_start(
                                t_out[t * 128:(t + 1) * 128, :],
                                eo[:, j, :])

            def allgather(l):
                nc.gpsimd.collective_compute(
                    "AllGather", mybir.AluOpType.bypass,
                    replica_groups=[[0, 1], [2, 3], [4, 5], [6, 7]],
                    ins=[d_tab_own[l][:].opt()],
                    outs=[d_tab_pair[l][:].opt()])

            transform0()
            allgather(0)
            for l in range(3):
                aggregate(l, FD[l])
                combine(l, FD[l])
                if l < 2:
                    allgather(l + 1)
    nc.compile()
    return nc


def _get_compiled(inputs):
    k = "prog"
    if k not in _CACHE:
        plan = _build_plan(inputs["edge_index"], inputs["edge_weight"])
        nc = _build_program(plan["nchunk"], plan["blk_of_sub"],
                            plan["start_of_sub"])
        _CACHE[k] = (plan, nc)
    return _CACHE[k]


def kernel(_trace=False, **inputs):
    from concourse.bass_utils import run_bass_kernel_spmd

    plan, nc = _get_compiled(inputs)

    x = np.asarray(inputs["x"], np.float32)
    xpad = np.zeros((NCORE, NOWN, IN_DIM), np.float32)
    for c in range(NCORE):
        xpad[c, :12500] = x[c * 12500:(c + 1) * 12500]

    def wchunks(W, n):
        out = np.zeros((n, 128, W.shape[1]), np.float32)
        for j in range(n):
            out[j, :min(128, W.shape[0] - j * 128)] = \
                W[j * 128:(j + 1) * 128]
        return out

    W0 = np.asarray(inputs["W0"], np.float32)
    P0 = np.asarray(inputs["P0w"], np.float32)
    W1p = wchunks(np.asarray(inputs["W1"], np.float32), 1)
    P1p = wchunks(np.asarray(inputs["P1w"], np.float32), 1)
    W2p = wchunks(np.asarray(inputs["W2"], np.float32), 1)
    b0r = np.tile((np.asarray(inputs["b0"]) + np.asarray(inputs["P0b"]))
                  .astype(np.float32), (128, 1))
    b1r = np.tile((np.asarray(inputs["b1"]) + np.asarray(inputs["P1b"]))
                  .astype(np.float32), (128, 1))
    b2r = np.tile(np.asarray(inputs["b2"]).astype(np.float32), (128, 1))

    in_maps = []
    for c in range(NCORE):
        in_maps.append({
            "x": xpad[c],
            "idx": plan["idx"][c], "msk": plan["msk"][c],
            "W0p": wchunks(W0, 2), "P0p": wchunks(P0, 2),
            "W1p": W1p, "P1p": P1p, "W2p": W2p,
            "b0r": b0r, "b1r": b1r, "b2r": b2r,
            "ident": np.eye(128, dtype=np.float32),
        })
    res = run_bass_kernel_spmd(nc, in_maps, core_ids=list(range(NCORE)),
                               trace=_trace)
    if _trace:
        kernel.last_exec_ns = res.exec_time_ns
    out = np.zeros((N, K), np.float32)
    for c in range(NCORE):
        out[c * 12500:(c + 1) * 12500] = res.results[c]["out"][:12500]
    return out
